# revision 1
# baseline (speedup 1.0000x reference)
"""TRN2 Bass kernel for nn_CommLayer (gnn message passing).

Math: x [B=65536, 512] viewed as [B, 8 agents, 64]; per agent a:
    y_a = tanh(x_a @ Wh.T + (sum_{a'!=a} x_{a'}) @ Wc.T / 7)
Equivalently y = tanh(x @ WT) with WT [512, 512]:
    WT[(a,d), (a',e)] = Wh[e,d] if a==a' else Wc[e,d]/7

Sharding: data-parallel over batch across 8 NeuronCores (8192 rows each);
WT replicated. Per core the kernel streams 128-row subtiles:
  - 4x PE transpose of x chunks into one PSUM bank ([128,512], f32r)
  - 1x DVE copy PSUM->SBUF (rounds to float32r)
  - 4x accumulating float32r matmuls (stationary = x^T chunk, moving =
    WT chunk rows, N=512) into a second PSUM bank
  - tanh on ScalarE from PSUM straight into the output staging tile
Transposes are emitted one subtile ahead of the matmuls so the PE never
stalls on the DVE copy. Input DMAs ride the sync queue, output DMAs the
scalar queue (avoids head-of-line blocking between groups). float32r
runs the PE at 1 cycle/row (vs 4 for fp32) at ~the accuracy of the PE's
own fp32 path.
"""
import sys

sys.path.insert(0, "/opt/trn_rl_repo")

import numpy as np

BATCH = 65536
D = 512
NAGENT = 8
DA = 64
NORM = NAGENT - 1
NCORES = 8
SHARD = BATCH // NCORES  # 8192
GROUP = 512              # rows per DMA group (1 MiB fp32)
NGROUP = SHARD // GROUP  # 16
SUBT = GROUP // 128      # 4 subtiles per group
NCHUNK = D // 128        # 4

_CACHE: dict = {}

# PRECISE=True switches the PE datapath from float32r (1 cyc/row, ~2.5e-3
# max rel err vs the fp32 reference) to float32 (4 cyc/row, ~6e-4) at a
# ~1.5x runtime cost. float32r noise is within ~4x of the PE's own fp32
# accumulation noise, so the fast path is the default.
PRECISE = False


def _build_nc():
    import concourse.mybir as mybir
    import concourse.tile as tile
    from concourse import bacc

    nc = bacc.Bacc("TRN2", target_bir_lowering=False, debug=False)

    f32 = mybir.dt.float32
    f32r = f32 if PRECISE else mybir.dt.float32r

    x_d = nc.dram_tensor("x", [SHARD, D], f32r, kind="ExternalInput")
    wt_d = nc.dram_tensor("wt", [D, D], f32r, kind="ExternalInput")
    id_d = nc.dram_tensor("ident", [128, 128], f32r, kind="ExternalInput")
    y_d = nc.dram_tensor("y", [SHARD, D], f32, kind="ExternalOutput")

    # row = g*GROUP + q*128 + p  ->  [g, p, q, f]
    xv = x_d[:].rearrange("(g q p) f -> g p q f", p=128, q=SUBT)
    yv = y_d[:].rearrange("(g q p) f -> g p q f", p=128, q=SUBT)
    wv = wt_d[:].rearrange("(c p) f -> p c f", p=128)

    NT = NGROUP * SUBT  # total subtiles

    with tile.TileContext(nc) as tc:
        with (
            tc.tile_pool(name="const", bufs=1) as const,
            tc.tile_pool(name="xg", bufs=5) as xgp,
            tc.tile_pool(name="og", bufs=4) as ogp,
            tc.tile_pool(name="xts", bufs=4) as xtsp,
            tc.tile_pool(name="pst", bufs=4, space="PSUM") as pstp,
            tc.tile_pool(name="psy", bufs=3, space="PSUM") as psyp,
        ):
            # queue layout: sync (HWDGE) = all input loads, gpsimd (SWDGE)
            # = all output stores, scalar = weights at t=0 then tanhs.
            # Loads never wait behind stores and vice versa.
            def in_eng(g):
                return nc.sync

            def out_eng(g):
                return nc.gpsimd

            # weights ride the scalar queue at t=0 (idle until tanhs start)
            wtile = const.tile([128, NCHUNK, D], f32r)
            nc.scalar.dma_start(wtile[:], wv)
            ident = const.tile([128, 128], f32r)
            nc.sync.dma_start(ident[:], id_d[:])

            xg_tiles = {}

            def load_group(g, split=False):
                xg = xgp.tile([128, SUBT, D], f32r, tag="xg", name=f"xg{g}")
                if split:
                    # per-subtile slices: first group so compute starts on
                    # slice 0, last group so its compute pipelines with the
                    # final DMAs instead of waiting on one 1 MiB sem.
                    # Group 0 races sync+gpsimd (stores idle then); later
                    # groups stay on sync (gpsimd is mid-store by then).
                    engs = [nc.sync, nc.gpsimd, nc.sync, nc.gpsimd] if g == 0 \
                        else [nc.sync] * SUBT
                    for q in range(SUBT):
                        engs[q].dma_start(xg[:, q, :], xv[g, :, q, :])
                else:
                    in_eng(g).dma_start(xg[:], xv[g, :, :, :])
                xg_tiles[g] = xg

            load_group(0, split=True)

            # stage 1 of subtile t: transposes + fused rounding copy
            def stage1(t):
                g, q = divmod(t, SUBT)
                if q == 0 and g + 1 < NGROUP:
                    load_group(g + 1, split=(g + 1 == NGROUP - 1))
                xg = xg_tiles[g]
                pst = pstp.tile([128, D], f32r, tag="pst")
                for c in range(NCHUNK):
                    nc.tensor.transpose(
                        pst[:, c * 128:(c + 1) * 128],
                        xg[:, q, c * 128:(c + 1) * 128],
                        ident[:],
                    )
                xt = xtsp.tile([128, D], f32r, tag="xts")
                nc.vector.tensor_copy(xt[:], pst[:])
                return xt

            # stage 2 of subtile t: matmuls + tanh (+ group store)
            og_tiles = {}

            def stage2(t, xt):
                g, q = divmod(t, SUBT)
                if q == 0:
                    og_tiles[g] = ogp.tile([128, SUBT, D], f32, tag="og", name=f"og{g}")
                og = og_tiles[g]
                psy = psyp.tile([128, D], f32, tag="psy")
                for c in range(NCHUNK):
                    nc.tensor.matmul(
                        psy[:],
                        xt[:, c * 128:(c + 1) * 128],
                        wtile[:, c, :],
                        start=(c == 0),
                        stop=(c == NCHUNK - 1),
                    )
                nc.scalar.activation(
                    og[:, q, :], psy[:], mybir.ActivationFunctionType.Tanh
                )
                # stores go to the opposite queue of this group's load;
                # the last two groups store per subtile so the tail drains
                # as tanhs retire instead of in one final burst
                if g >= NGROUP - 2:
                    out_eng(g).dma_start(yv[g, :, q, :], og[:, q, :])
                elif q == SUBT - 1:
                    out_eng(g).dma_start(yv[g, :, :, :], og[:])

            # software pipeline: transposes run one subtile ahead
            prev = stage1(0)
            for t in range(NT):
                nxt = stage1(t + 1) if t + 1 < NT else None
                stage2(t, prev)
                prev = nxt

    nc.compile()
    return nc


def _get_nc():
    if "nc" not in _CACHE:
        _CACHE["nc"] = _build_nc()
    return _CACHE["nc"]


def _build_wt(hw: np.ndarray, cw: np.ndarray) -> np.ndarray:
    wt = np.empty((D, D), dtype=np.float32)
    whT = np.ascontiguousarray(hw.T)
    wcT = np.ascontiguousarray(cw.T) / np.float32(NORM)
    for a in range(NAGENT):
        for a2 in range(NAGENT):
            blk = whT if a == a2 else wcT
            wt[a * DA:(a + 1) * DA, a2 * DA:(a2 + 1) * DA] = blk
    return wt


def kernel(**inputs) -> np.ndarray:
    from concourse.bass_utils import run_bass_kernel_spmd

    x = np.ascontiguousarray(np.asarray(inputs["x"], dtype=np.float32))
    hw = np.asarray(inputs["hidden_weights"], dtype=np.float32)
    cw = np.asarray(inputs["communication_weights"], dtype=np.float32)
    assert x.shape == (BATCH, D), x.shape

    wt = _build_wt(hw, cw)
    ident = np.eye(128, dtype=np.float32)

    nc = _get_nc()
    shards = x.reshape(NCORES, SHARD, D)
    in_maps = [
        {"x": np.ascontiguousarray(shards[i]), "wt": wt, "ident": ident}
        for i in range(NCORES)
    ]
    res = run_bass_kernel_spmd(nc, in_maps, core_ids=list(range(NCORES)))
    y = np.concatenate([r["y"] for r in res.results], axis=0)
    return y.astype(np.float32, copy=False)

